# revision 17
# baseline (speedup 1.0000x reference)
"""Trainium2 Bass kernel for the ChipletThermalModel problem.

Math per chiplet i, per grid point (summed over 16 chiplets), after
normalizing by `a` (F(a,b,c) = a*F(1, b/a, c/a), so a^2 -> 1 and the
overall factor a folds into the per-chiplet scale):
  b'± = pb± ± x/(a*lx),  c'± = pc± ± y/(a*ly)       (pb,pc host-precomputed)
  For the 4 sign combos k=(s1,s2):
    rδ_k = AbsRsqrt(s0_k + 1) = 1/δ_k   (ACT table op, ~4e-5 rel err)
    δ_k  = (s0_k + 1) * rδ_k            (one DVE STT)
  b-side (pair-merged over s2):
    lnq_b(s1) = ln(1+b'²) - ln((c'm+δ_{s1m})(c'p+δ_{s1p}))
    contribution -= s_i * b'_{s1} * lnq_b(s1)        (s_i = Pi*A*a*2/√π)
  c-side symmetric; atan side: contribution -= s_i * atan(b'c' * rδ_k)
  (the HW ISA has no tensor-tensor divide; recip-approx custom DVE ops
  measure 3.1us/op, AbsRsqrt on ACT is far cheaper) plus endC=A*B_off*ΣPi.

The -s_i scale is applied by the accumulating PE matmul with a diagonal
stationary matrix diag(-s_i[p]) (host-precomputed per partition), so no
elementwise scale ops remain.

Engine split per [128,1024] fp32 tile per chiplet, using MEASURED HW
throughputs (Pool mult 5.8us -> Pool gets only add/sub; DVE TT 1.20us,
DVE TS 1.00us, Pool add ~2.0us, ACT ~1.43us):
  ACT : 18 ops in 3 table phases (4 AbsRsqrt | 4+4 Ln | 4 Atan) + 2
        Square (in every table -> no extra load) + PSUM eviction.
  DVE : 4 prep tensor_scalar + 2 sq + 4 δ-STT + 16 mult TT.
  Pool: 16 add/sub TT.
  PE  : 16 matmul-halves [128,512] fp32 accumulating into PSUM.

Sharding: batch dim (64) split across 8 cores -> 8 rows/core, laid out as
[128 partitions, 4096]; per-(batch-row,chiplet) parameters become
per-partition [128,1] scalars and [128,128] diag blocks (host-precomputed).
"""
import sys
import numpy as np

for _p in ("/opt/trn_rl_repo",):
    if _p not in sys.path:
        sys.path.insert(0, _p)

N_CORES = 8
B, NCHIP, G2 = 64, 16, 65536
RPC = B // N_CORES            # batch rows per core = 8
P = 128                       # SBUF partitions
F = RPC * G2 // P             # free-dim columns per core = 4096
W = 1024                      # columns per processing group
NG = F // W                   # groups
WK_BUFS = 40                  # work-tile ring size
REP = P // RPC                # partitions per batch row = 16
NPAR = 4 * NCHIP + 1          # params columns (4 per chiplet + endC)
C1 = float(2.0 / np.sqrt(np.pi))

# Which of the flexible elementwise ops run on DVE ("d") vs Pool ("p") vs
# ACT ("a", unary-capable ops only). Lists give per-index picks.
# Measured HW: Pool can only afford adds/subs; all mults go to DVE/ACT.
ENG = {
    "sq": ["d", "d", "a", "a"],          # b'm², b'p², c'm², c'p²
    "s0": "p",
    "pd": ["d", "d", "p", "p", "p", "p", "p", "p"],
    "prod": "d", "sub": "p", "bc": "d", "targ": "d", "blnq": "d",
}


def _make_schedule(mode):
    """Emission order over (stage, chiplet). Engines dispatch in order with
    head-of-line blocking; pair-blocked ACT phases keep table loads at 3
    per chiplet pair, and hoisting the next pair's early stages keeps the
    vector engines fed while ACT chews a phase."""
    s = []
    if mode == "pairs":
        for pr in range(NCHIP // 2):
            i0, i1 = 2 * pr, 2 * pr + 1
            s += [(0, i0), (1, i0), (0, i1), (1, i1),
                  (2, i0), (2, i1), (3, i0), (3, i1),
                  (4, i0), (4, i1), (5, i0), (5, i1),
                  (6, i0), (6, i1), (7, i0), (7, i1)]
    elif mode == "sw":
        # software-pipelined: next pair's prep/squares emitted between this
        # pair's ln phase and blnq stage
        s += [(0, 0), (1, 0), (0, 1), (1, 1)]
        for pr in range(NCHIP // 2):
            i0, i1 = 2 * pr, 2 * pr + 1
            j0, j1 = i0 + 2, i1 + 2
            s += [(2, i0), (2, i1), (3, i0), (3, i1),
                  (4, i0), (4, i1),
                  (0, j0), (1, j0), (0, j1), (1, j1),
                  (5, i0), (5, i1),
                  (6, i0), (6, i1), (7, i0), (7, i1)]
    elif mode == "sw4":
        # deep rotation: every ACT phase consumes inputs produced a full
        # half-iteration earlier. Steady state works on pair p's tail
        # (ln/blnq/atan) while building pair q=p+1 from scratch.
        s += [(0, 0), (1, 0), (0, 1), (1, 1),
              (2, 0), (2, 1), (3, 0), (3, 1)]
        for pr in range(NCHIP // 2):
            i0, i1 = 2 * pr, 2 * pr + 1
            j0, j1 = i0 + 2, i1 + 2
            s += [(4, i0), (4, i1),
                  (0, j0), (1, j0), (0, j1), (1, j1),
                  (2, j0), (2, j1),
                  (3, j0), (3, j1),
                  (5, i0), (5, i1),
                  (6, i0), (6, i1), (7, i0), (7, i1)]
    return [t for t in s if 0 <= t[1] < NCHIP]


SCHEDULE = _make_schedule("sw4")


def _build_program(scal):
    """Build the Bass program. `scal` holds python-float per-chiplet scalars."""
    from concourse import bacc, tile
    import concourse.mybir as mybir
    import bass_rust as _bass_rust

    AF = mybir.ActivationFunctionType
    OP = mybir.AluOpType
    FP32 = mybir.dt.float32

    nc = bacc.Bacc("TRN2", target_bir_lowering=False, debug=False,
                   enable_asserts=False)

    # Pin the ACT instruction order with scheduler-only (nosync) dep edges:
    # the engine is in-order so same-engine ordering costs nothing at
    # runtime, but it stops the list scheduler from interleaving ops of
    # different table phases (table thrash).
    _act_prev = [None]

    def _act(out, in_, func, **kw):
        inst = nc.scalar.activation(out, in_, func, **kw)
        if _act_prev[0] is not None:
            _bass_rust.add_dep_helper(inst.ins, _act_prev[0], sync=False,
                                      reason="act table phase order")
        _act_prev[0] = inst.ins
        return inst

    xin = nc.dram_tensor("xin", [P, F], FP32, kind="ExternalInput")
    yin = nc.dram_tensor("yin", [P, F], FP32, kind="ExternalInput")
    prm = nc.dram_tensor("prm", [P, NPAR], FP32, kind="ExternalInput")
    dgf = nc.dram_tensor("dgf", [P, NCHIP * P], FP32, kind="ExternalInput")
    out = nc.dram_tensor("out", [P, F], FP32, kind="ExternalOutput")

    inv_la = scal["inv_la"]
    inv_ha = scal["inv_ha"]

    MP = ("m", "p")
    HALF = W // 2

    def eng(which):
        return nc.vector if which == "d" else nc.gpsimd

    with tile.TileContext(nc) as tc:
        with tc.tile_pool(name="cst", bufs=1) as cst, \
             tc.tile_pool(name="io", bufs=2) as io, \
             tc.tile_pool(name="ps", bufs=2, space="PSUM") as ps, \
             tc.tile_pool(name="wk", bufs=WK_BUFS) as wk:
            prmt = cst.tile([P, NPAR], FP32)
            nc.sync.dma_start(prmt[:], prm[:])
            dgt = cst.tile([P, NCHIP * P], FP32)
            nc.sync.dma_start(dgt[:], dgf[:])

            def pcol(i, k):           # [128,1] per-partition param AP
                return prmt[:, 4 * i + k: 4 * i + k + 1]

            endC = prmt[:, 4 * NCHIP: 4 * NCHIP + 1]

            for g in range(NG):
                cs = slice(g * W, (g + 1) * W)
                xt = io.tile([P, W], FP32, tag="xt")
                yt = io.tile([P, W], FP32, tag="yt")
                res = io.tile([P, W], FP32, tag="res")
                nc.sync.dma_start(xt[:], xin[:, cs])
                nc.sync.dma_start(yt[:], yin[:, cs])
                # PSUM accumulators: one bank per half-tile (matmul moving
                # free dim is capped at 512)
                acc = [ps.tile([P, HALF], FP32, tag=f"acc{h}",
                               name=f"acc{h}")
                       for h in range(W // HALF)]
                mm_count = [0]
                MM_TOTAL = NCHIP * 8 * (W // HALF)

                def accum(t, i):
                    """res_psum += diag(-s_i) @ t on the PE."""
                    stat = dgt[:, i * P:(i + 1) * P]
                    for h, a_ in enumerate(acc):
                        first = mm_count[0] < len(acc)
                        last = mm_count[0] >= MM_TOTAL - len(acc)
                        nc.tensor.matmul(
                            a_[:], stat, t[:, h * HALF:(h + 1) * HALF],
                            start=first, stop=last)
                        mm_count[0] += 1

                def wtile(nm):
                    return wk.tile([P, W], FP32, tag="wk", name=nm)

                # per-chiplet state dicts, keyed by chiplet index
                st = [dict() for _ in range(NCHIP)]

                def st0(i):
                    """b'±, c'± via DVE tensor_scalar (x*(±1/(a*l)) + p)."""
                    e = st[i]
                    bs, cs_ = {}, {}
                    for k, sgn, col in (("m", -1.0, 0), ("p", 1.0, 1)):
                        t = wtile("b" + k)
                        nc.vector.tensor_scalar(t[:], xt[:], sgn * inv_la[i],
                                                pcol(i, col), OP.mult, OP.add)
                        bs[k] = t
                    for k, sgn, col in (("m", -1.0, 2), ("p", 1.0, 3)):
                        t = wtile("c" + k)
                        nc.vector.tensor_scalar(t[:], yt[:], sgn * inv_ha[i],
                                                pcol(i, col), OP.mult, OP.add)
                        cs_[k] = t
                    e.update(b=bs, c=cs_)

                def st1(i):
                    """squares and s0 = b'² + c'² per combo."""
                    e = st[i]
                    sqb, sqc = {}, {}
                    srcs = [("b", e["b"]), ("c", e["c"])]
                    sqi = 0
                    for nm, src in srcs:
                        d = sqb if nm == "b" else sqc
                        for k in MP:
                            t = wtile("sq" + nm)
                            which = ENG["sq"][sqi]
                            sqi += 1
                            if which == "a":
                                _act(t[:], src[k][:], AF.Square)
                            else:
                                eng(which).tensor_tensor(
                                    t[:], src[k][:], src[k][:], OP.mult)
                            d[k] = t
                    s0 = {}
                    for kx in MP:
                        for ky in MP:
                            t = wtile("s0")
                            eng(ENG["s0"]).tensor_tensor(
                                t[:], sqb[kx][:], sqc[ky][:], OP.add)
                            s0[kx + ky] = t
                    e.update(sqb=sqb, sqc=sqc, s0=s0)

                def st2(i):
                    """ACT rsqrt phase: rδ = AbsRsqrt(s0+1); δ = (s0+1)*rδ
                    (DVE STT, in place onto s0)."""
                    e = st[i]
                    rd, dl = {}, {}
                    for kk in ("mm", "mp", "pm", "pp"):
                        t = wtile("rd")
                        _act(t[:], e["s0"][kk][:], AF.Abs_reciprocal_sqrt,
                             bias=1.0)
                        rd[kk] = t
                    for kk in ("mm", "mp", "pm", "pp"):
                        t = e["s0"][kk]      # in place: (s0+1)*rδ -> δ
                        nc.vector.scalar_tensor_tensor(
                            t[:], t[:], 1.0, rd[kk][:], OP.add, OP.mult)
                        dl[kk] = t
                    e.update(rd=rd, dl=dl)

                def st3(i):
                    """pd sums, pair products, bc, targ = bc*rδ (DVE+Pool)."""
                    e = st[i]
                    bs, cs_, dl, rd = e["b"], e["c"], e["dl"], e["rd"]
                    cpd, bpd = {}, {}
                    pdi = 0
                    for kx in MP:
                        for ky in MP:
                            kk = kx + ky
                            t = wtile("cpd")
                            eng(ENG["pd"][pdi]).tensor_tensor(
                                t[:], cs_[ky][:], dl[kk][:], OP.add)
                            cpd[kk] = t
                            pdi += 1
                    for kx in MP:
                        for ky in MP:
                            kk = kx + ky
                            t = dl[kk]           # δ dead after bpd
                            eng(ENG["pd"][pdi]).tensor_tensor(
                                t[:], bs[kx][:], dl[kk][:], OP.add)
                            bpd[kk] = t
                            pdi += 1
                    # pair products; write onto one of the inputs
                    prod = {}
                    for kx in MP:      # b-side: fixed s1, product over s2
                        t = cpd[kx + "m"]
                        eng(ENG["prod"]).tensor_tensor(
                            t[:], cpd[kx + "m"][:], cpd[kx + "p"][:],
                            OP.mult)
                        prod["b" + kx] = t
                    for ky in MP:      # c-side: fixed s2, product over s1
                        t = bpd["m" + ky]
                        eng(ENG["prod"]).tensor_tensor(
                            t[:], bpd["m" + ky][:], bpd["p" + ky][:],
                            OP.mult)
                        prod["c" + ky] = t
                    # atan args: bc fresh, targ = bc*rδ in place on bc
                    targ = {}
                    for kx in MP:
                        for ky in MP:
                            kk = kx + ky
                            t = wtile("bc")
                            eng(ENG["bc"]).tensor_tensor(
                                t[:], bs[kx][:], cs_[ky][:], OP.mult)
                            eng(ENG["targ"]).tensor_tensor(
                                t[:], t[:], rd[kk][:], OP.mult)
                            targ[kk] = t
                    e.update(prod=prod, targ=targ)

                def st4(i):
                    """ACT ln phase: lax = Ln(sq+1) onto sq, lnp = Ln(prod)
                    onto prod."""
                    e = st[i]
                    lax, lnp = {}, {}
                    for sd, sq in (("b", e["sqb"]), ("c", e["sqc"])):
                        for k in MP:
                            t = sq[k]
                            _act(t[:], t[:], AF.Ln, bias=1.0)
                            lax[sd + k] = t
                    for sk, t in e["prod"].items():
                        _act(t[:], t[:], AF.Ln)
                        lnp[sk] = t
                    e.update(lax=lax, lnp=lnp)

                def st5(i):
                    """lnq = lax - lnp; blnq = b'*lnq; accumulate with
                    diag(-s_i)."""
                    e = st[i]
                    for sd, op in (("b", e["b"]), ("c", e["c"])):
                        for k in MP:
                            t = e["lax"][sd + k]
                            eng(ENG["sub"]).tensor_tensor(
                                t[:], t[:], e["lnp"][sd + k][:], OP.subtract)
                            eng(ENG["blnq"]).tensor_tensor(
                                t[:], op[k][:], t[:], OP.mult)
                            accum(t, i)

                def st6(i):
                    """ACT atan phase, in place on targ."""
                    e = st[i]
                    for kk, t in e["targ"].items():
                        _act(t[:], t[:], AF.Arctan)

                def st7(i):
                    """Accumulate the 4 atan tiles with diag(-s_i)."""
                    e = st[i]
                    for kk, t in e["targ"].items():
                        accum(t, i)
                    st[i] = {}   # drop tile refs

                stages = [st0, st1, st2, st3, st4, st5, st6, st7]
                for step, i in SCHEDULE:
                    stages[step](i)
                # evict PSUM -> SBUF (+endC) on ACT (Identity is in every
                # table -> no table load), then DMA out
                for h in range(W // HALF):
                    _act(res[:, h * HALF:(h + 1) * HALF], acc[h][:],
                         AF.Identity, bias=endC)
                nc.sync.dma_start(out[:, cs], res[:])
    nc.finalize()
    return nc


def _host_params(cx, cy, w, h, Pw, A, a, B_off, lx, ly, rows):
    """Per-core [128, NPAR] parameter matrix (per-partition scalars)."""
    pr = np.zeros((P, NPAR), dtype=np.float32)
    for i in range(NCHIP):
        la = a * lx[i]
        ha = a * ly[i]
        W0 = 0.5 * w[rows, i] / la
        H0 = 0.5 * h[rows, i] / ha
        cxl = cx[rows, i] / la
        cyl = cy[rows, i] / ha
        pr[:, 4 * i + 0] = np.repeat(W0 + cxl, REP)   # b'm = -x/(a lx) + .
        pr[:, 4 * i + 1] = np.repeat(W0 - cxl, REP)   # b'p = +x/(a lx) + .
        pr[:, 4 * i + 2] = np.repeat(H0 + cyl, REP)
        pr[:, 4 * i + 3] = np.repeat(H0 - cyl, REP)
    pr[:, 4 * NCHIP] = np.repeat(A * B_off * Pw[rows].sum(axis=1), REP)
    return np.ascontiguousarray(pr, dtype=np.float32)


def _host_diag(Pw, A, a, rows):
    """[128, NCHIP*128] fp32: per chiplet a diag(-Pi*A*a*2/sqrt(pi))."""
    dg = np.zeros((P, NCHIP * P), dtype=np.float32)
    idx = np.arange(P)
    for i in range(NCHIP):
        s = np.repeat(-C1 * A * a * Pw[rows, i], REP).astype(np.float32)
        dg[idx, i * P + idx] = s
    return np.ascontiguousarray(dg)


_CACHE = {}


def _get_executor(scal):
    """Build (once) the Bass program and a cached jitted SPMD callable.

    The callable takes the device-resident inputs as [8·128, ...] arrays
    sharded over 8 cores; the output scratch buffer is created on-device
    inside the same jit call (no host transfer)."""
    if "exec" in _CACHE:
        return _CACHE["exec"]

    import jax
    import jax.numpy as jnp
    from jax.sharding import Mesh, NamedSharding, PartitionSpec
    from jax.experimental.shard_map import shard_map
    from concourse import bass2jax
    import concourse.mybir as mybir

    nc = _build_program(scal)
    _CACHE["nc"] = nc
    bass2jax.install_neuronx_cc_hook()

    partition_name = (nc.partition_id_tensor.name
                      if nc.partition_id_tensor else None)
    in_names, out_names, out_avals = [], [], []
    for alloc in nc.m.functions[0].allocations:
        if not isinstance(alloc, mybir.MemoryLocationSet):
            continue
        name = alloc.memorylocations[0].name
        if alloc.kind == "ExternalInput":
            if name != partition_name:
                in_names.append(name)
        elif alloc.kind == "ExternalOutput":
            out_names.append(name)
            out_avals.append(jax.core.ShapedArray(
                tuple(alloc.tensor_shape), mybir.dt.np(alloc.dtype)))
    n_params = len(in_names)
    all_names = in_names + out_names
    if partition_name is not None:
        all_names = all_names + [partition_name]

    def _body(*args):
        # args = real inputs + one dummy buffer per output. On this
        # (axon/PJRT) path the output operands are inert: the NEFF rename
        # binds the bass "out" tensor to the custom-call RESULT buffer, so
        # the dummy is never read -- it only satisfies the hook's
        # param-order check. One persistent buffer is reused every call.
        operands = list(args)
        if partition_name is not None:
            operands.append(bass2jax.partition_id_tensor())
        outs = bass2jax._bass_exec_p.bind(
            *operands,
            out_avals=tuple(out_avals),
            in_names=tuple(all_names),
            out_names=tuple(out_names),
            lowering_input_output_aliases=(),
            sim_require_finite=True,
            sim_require_nnan=True,
            nc=nc,
        )
        return tuple(outs)

    devices = jax.devices()[:N_CORES]
    mesh = Mesh(np.asarray(devices), ("core",))
    sharding = NamedSharding(mesh, PartitionSpec("core"))
    sharded = jax.jit(
        shard_map(_body, mesh=mesh,
                  in_specs=(PartitionSpec("core"),) * (n_params + len(out_avals)),
                  out_specs=(PartitionSpec("core"),) * len(out_avals),
                  check_rep=False),
        keep_unused=True)

    # device-side dummy output operands (content never read)
    zshapes = [(N_CORES * s.shape[0], *s.shape[1:]) for s in out_avals]
    zdtypes = [s.dtype for s in out_avals]

    def _mk(shape_dtype):
        shape, dtype = shape_dtype
        return jax.jit(lambda: jnp.zeros(shape, dtype),
                       out_shardings=sharding)

    zeros_fns = [_mk(sd) for sd in zip(zshapes, zdtypes)]
    ex = {"sharded": sharded, "in_names": in_names, "zeros_fns": zeros_fns,
          "sharding": sharding, "n_params": n_params}
    _CACHE["exec"] = ex
    return ex


def _scal_from_inputs(a, lx, ly):
    af = float(np.asarray(a).reshape(-1)[0])
    lxf = np.asarray(lx, dtype=np.float64)
    lyf = np.asarray(ly, dtype=np.float64)
    return {
        "inv_la": [float(1.0 / (af * lxf[i])) for i in range(NCHIP)],
        "inv_ha": [float(1.0 / (af * lyf[i])) for i in range(NCHIP)],
    }


def _device_inputs(x, y, chiplets_x, chiplets_y, chiplets_width,
                   chiplets_height, chiplets_power, A, a, B_off, lx, ly):
    """Full-input -> per-core-stacked device arrays keyed by tensor name."""
    x = np.asarray(x, dtype=np.float32)
    y = np.asarray(y, dtype=np.float32)
    cx = np.asarray(chiplets_x, dtype=np.float32)
    cy = np.asarray(chiplets_y, dtype=np.float32)
    w = np.asarray(chiplets_width, dtype=np.float32)
    h = np.asarray(chiplets_height, dtype=np.float32)
    Pw = np.asarray(chiplets_power, dtype=np.float32)
    Af = float(np.asarray(A).reshape(-1)[0])
    af = float(np.asarray(a).reshape(-1)[0])
    Bf = float(np.asarray(B_off).reshape(-1)[0])
    lxf = np.asarray(lx, dtype=np.float64)
    lyf = np.asarray(ly, dtype=np.float64)

    xs = np.ascontiguousarray(x.reshape(N_CORES * P, F))
    ys = np.ascontiguousarray(y.reshape(N_CORES * P, F))
    prs = np.concatenate(
        [_host_params(cx, cy, w, h, Pw, Af, af, Bf, lxf, lyf,
                      slice(c * RPC, (c + 1) * RPC)) for c in range(N_CORES)],
        axis=0)
    dgs = np.concatenate(
        [_host_diag(Pw, Af, af, slice(c * RPC, (c + 1) * RPC))
         for c in range(N_CORES)], axis=0)
    return {"xin": xs, "yin": ys, "prm": prs, "dgf": dgs}


def run(x, y, chiplets_x, chiplets_y, chiplets_width, chiplets_height,
        chiplets_power, A, a, B_off, lx, ly, grid=None):
    import jax

    ex = _get_executor(_scal_from_inputs(a, lx, ly))
    arrs = _device_inputs(x, y, chiplets_x, chiplets_y, chiplets_width,
                          chiplets_height, chiplets_power, A, a, B_off,
                          lx, ly)
    ins = [jax.device_put(arrs[nm], ex["sharding"]) for nm in ex["in_names"]]
    scratch = [zf() for zf in ex["zeros_fns"]]
    out = ex["sharded"](*ins, *scratch)
    full = np.asarray(out[0]).reshape(B, G2).astype(np.float32, copy=False)
    return full


def kernel(**inputs):
    return run(**inputs)


# revision 27
# speedup vs baseline: 1.0058x; 1.0058x over previous
"""Trainium2 Bass kernel for the ChipletThermalModel problem.

Math per chiplet i, per grid point (summed over 16 chiplets), after
normalizing by `a` (F(a,b,c) = a*F(1, b/a, c/a), so a^2 -> 1 and the
overall factor a folds into the per-chiplet scale):
  b'± = pb± ± x/(a*lx),  c'± = pc± ± y/(a*ly)       (pb,pc host-precomputed)
  For the 4 sign combos k=(s1,s2):
    rδ_k = AbsRsqrt(s0_k + 1) = 1/δ_k   (ACT table op, ~4e-5 rel err)
    δ_k  = (s0_k + 1) * rδ_k            (one DVE STT)
  b-side (pair-merged over s2):
    lnq_b(s1) = ln(1+b'²) - ln((c'm+δ_{s1m})(c'p+δ_{s1p}))
    contribution -= s_i * b'_{s1} * lnq_b(s1)        (s_i = Pi*A*a*2/√π)
  c-side symmetric; atan side: contribution -= s_i * atan(b'c' * rδ_k)
  (the HW ISA has no tensor-tensor divide; recip-approx custom DVE ops
  measure 3.1us/op, AbsRsqrt on ACT is far cheaper) plus endC=A*B_off*ΣPi.

The -s_i scale is applied by the accumulating PE matmul with a diagonal
stationary matrix diag(-s_i[p]) (host-precomputed per partition), so no
elementwise scale ops remain.

Engine split per [128,1024] fp32 tile per chiplet, using MEASURED HW
throughputs (Pool mult 5.8us -> Pool gets only add/sub; DVE TT 1.20us,
DVE TS 1.00us, Pool add ~2.0us, ACT ~1.43us):
  ACT : 18 ops in 3 table phases (4 AbsRsqrt | 4+4 Ln | 4 Atan) + 2
        Square (in every table -> no extra load) + PSUM eviction.
  DVE : 4 prep tensor_scalar + 2 sq + 4 δ-STT + 16 mult TT.
  Pool: 16 add/sub TT.
  PE  : 16 matmul-halves [128,512] fp32 accumulating into PSUM.

Sharding: batch dim (64) split across 8 cores -> 8 rows/core, laid out as
[128 partitions, 4096]; per-(batch-row,chiplet) parameters become
per-partition [128,1] scalars and [128,128] diag blocks (host-precomputed).
"""
import sys
import numpy as np

for _p in ("/opt/trn_rl_repo",):
    if _p not in sys.path:
        sys.path.insert(0, _p)

N_CORES = 8
B, NCHIP, G2 = 64, 16, 65536
RPC = B // N_CORES            # batch rows per core = 8
P = 128                       # SBUF partitions
F = RPC * G2 // P             # free-dim columns per core = 4096
W = 1024                      # columns per processing group
NG = F // W                   # groups
WK_BUFS = 40                  # work-tile ring size
REP = P // RPC                # partitions per batch row = 16
NPAR = 4 * NCHIP + 1          # params columns (4 per chiplet + endC)
C1 = float(2.0 / np.sqrt(np.pi))

# Which of the flexible elementwise ops run on DVE ("d") vs Pool ("p") vs
# ACT ("a", unary-capable ops only). Lists give per-index picks.
# Measured HW: Pool can only afford adds/subs; all mults go to DVE/ACT.
ENG = {
    "sq": ["d", "d", "a", "a"],          # b'm², b'p², c'm², c'p²
    "s0": "p",
    "pd": ["d", "d", "p", "p", "p", "p", "p", "p"],
    "prod": "d", "sub": "p", "bc": "d", "targ": "d", "blnq": "d",
}


def _make_schedule(mode):
    """Emission order over (stage, chiplet). Engines dispatch in order with
    head-of-line blocking; pair-blocked ACT phases keep table loads at 3
    per chiplet pair, and hoisting the next pair's early stages keeps the
    vector engines fed while ACT chews a phase."""
    s = []
    if mode == "pairs":
        for pr in range(NCHIP // 2):
            i0, i1 = 2 * pr, 2 * pr + 1
            s += [(0, i0), (1, i0), (0, i1), (1, i1),
                  (2, i0), (2, i1), (3, i0), (3, i1),
                  (4, i0), (4, i1), (5, i0), (5, i1),
                  (6, i0), (6, i1), (7, i0), (7, i1)]
    elif mode == "sw":
        # software-pipelined: next pair's prep/squares emitted between this
        # pair's ln phase and blnq stage
        s += [(0, 0), (1, 0), (0, 1), (1, 1)]
        for pr in range(NCHIP // 2):
            i0, i1 = 2 * pr, 2 * pr + 1
            j0, j1 = i0 + 2, i1 + 2
            s += [(2, i0), (2, i1), (3, i0), (3, i1),
                  (4, i0), (4, i1),
                  (0, j0), (1, j0), (0, j1), (1, j1),
                  (5, i0), (5, i1),
                  (6, i0), (6, i1), (7, i0), (7, i1)]
    elif mode == "sw4":
        # deep rotation: every ACT phase consumes inputs produced a full
        # half-iteration earlier. Steady state works on pair p's tail
        # (ln/blnq/atan) while building pair q=p+1 from scratch.
        s += [(0, 0), (1, 0), (0, 1), (1, 1),
              (2, 0), (2, 1), (3, 0), (3, 1)]
        for pr in range(NCHIP // 2):
            i0, i1 = 2 * pr, 2 * pr + 1
            j0, j1 = i0 + 2, i1 + 2
            s += [(4, i0), (4, i1),
                  (0, j0), (1, j0), (0, j1), (1, j1),
                  (2, j0), (2, j1),
                  (3, j0), (3, j1),
                  (5, i0), (5, i1),
                  (6, i0), (6, i1), (7, i0), (7, i1)]
    return [t for t in s if 0 <= t[1] < NCHIP]


SCHEDULE = _make_schedule("sw4")


def _build_program(scal):
    """Build the Bass program. `scal` holds python-float per-chiplet scalars."""
    from concourse import bacc, tile
    import concourse.mybir as mybir
    import bass_rust as _bass_rust

    AF = mybir.ActivationFunctionType
    OP = mybir.AluOpType
    FP32 = mybir.dt.float32

    nc = bacc.Bacc("TRN2", target_bir_lowering=False, debug=False,
                   enable_asserts=False)

    # Pin the ACT instruction order with scheduler-only (nosync) dep edges:
    # the engine is in-order so same-engine ordering costs nothing at
    # runtime, but it stops the list scheduler from interleaving ops of
    # different table phases (table thrash).
    _act_prev = [None]

    def _act(out, in_, func, **kw):
        inst = nc.scalar.activation(out, in_, func, **kw)
        if _act_prev[0] is not None:
            _bass_rust.add_dep_helper(inst.ins, _act_prev[0], sync=False,
                                      reason="act table phase order")
        _act_prev[0] = inst.ins
        return inst

    xin = nc.dram_tensor("xin", [P, F], FP32, kind="ExternalInput")
    yin = nc.dram_tensor("yin", [P, F], FP32, kind="ExternalInput")
    prm = nc.dram_tensor("prm", [P, NPAR], FP32, kind="ExternalInput")
    dgf = nc.dram_tensor("dgf", [P, NCHIP * P], FP32, kind="ExternalInput")
    out = nc.dram_tensor("out", [P, F], FP32, kind="ExternalOutput")

    inv_la = scal["inv_la"]
    inv_ha = scal["inv_ha"]

    MP = ("m", "p")
    HALF = W // 2

    def eng(which):
        return nc.vector if which == "d" else nc.gpsimd

    with tile.TileContext(nc) as tc:
        with tc.tile_pool(name="cst", bufs=1) as cst, \
             tc.tile_pool(name="io", bufs=2) as io, \
             tc.tile_pool(name="ps", bufs=2, space="PSUM") as ps, \
             tc.tile_pool(name="wk", bufs=WK_BUFS) as wk:
            prmt = cst.tile([P, NPAR], FP32)
            nc.sync.dma_start(prmt[:], prm[:])
            dgt = cst.tile([P, NCHIP * P], FP32)
            nc.sync.dma_start(dgt[:], dgf[:])

            def pcol(i, k):           # [128,1] per-partition param AP
                return prmt[:, 4 * i + k: 4 * i + k + 1]

            endC = prmt[:, 4 * NCHIP: 4 * NCHIP + 1]

            for g in range(NG):
                cs = slice(g * W, (g + 1) * W)
                xt = io.tile([P, W], FP32, tag="xt")
                yt = io.tile([P, W], FP32, tag="yt")
                res = io.tile([P, W], FP32, tag="res")
                nc.sync.dma_start(xt[:], xin[:, cs])
                nc.sync.dma_start(yt[:], yin[:, cs])
                # PSUM accumulators: one bank per half-tile (matmul moving
                # free dim is capped at 512)
                acc = [ps.tile([P, HALF], FP32, tag=f"acc{h}",
                               name=f"acc{h}")
                       for h in range(W // HALF)]
                mm_count = [0]
                MM_TOTAL = NCHIP * 8 * (W // HALF)

                def accum(t, i):
                    """res_psum += diag(-s_i) @ t on the PE."""
                    stat = dgt[:, i * P:(i + 1) * P]
                    for h, a_ in enumerate(acc):
                        first = mm_count[0] < len(acc)
                        last = mm_count[0] >= MM_TOTAL - len(acc)
                        nc.tensor.matmul(
                            a_[:], stat, t[:, h * HALF:(h + 1) * HALF],
                            start=first, stop=last)
                        mm_count[0] += 1

                def wtile(nm):
                    return wk.tile([P, W], FP32, tag="wk", name=nm)

                # per-chiplet state dicts, keyed by chiplet index
                st = [dict() for _ in range(NCHIP)]

                def st0(i):
                    """b'±, c'± via DVE tensor_scalar (x*(±1/(a*l)) + p)."""
                    e = st[i]
                    bs, cs_ = {}, {}
                    for k, sgn, col in (("m", -1.0, 0), ("p", 1.0, 1)):
                        t = wtile("b" + k)
                        nc.vector.tensor_scalar(t[:], xt[:], sgn * inv_la[i],
                                                pcol(i, col), OP.mult, OP.add)
                        bs[k] = t
                    for k, sgn, col in (("m", -1.0, 2), ("p", 1.0, 3)):
                        t = wtile("c" + k)
                        nc.vector.tensor_scalar(t[:], yt[:], sgn * inv_ha[i],
                                                pcol(i, col), OP.mult, OP.add)
                        cs_[k] = t
                    e.update(b=bs, c=cs_)

                def st1(i):
                    """squares and s0 = b'² + c'² per combo."""
                    e = st[i]
                    sqb, sqc = {}, {}
                    srcs = [("b", e["b"]), ("c", e["c"])]
                    sqi = 0
                    for nm, src in srcs:
                        d = sqb if nm == "b" else sqc
                        for k in MP:
                            t = wtile("sq" + nm)
                            which = ENG["sq"][sqi]
                            sqi += 1
                            if which == "a":
                                _act(t[:], src[k][:], AF.Square)
                            else:
                                eng(which).tensor_tensor(
                                    t[:], src[k][:], src[k][:], OP.mult)
                            d[k] = t
                    s0 = {}
                    for kx in MP:
                        for ky in MP:
                            t = wtile("s0")
                            eng(ENG["s0"]).tensor_tensor(
                                t[:], sqb[kx][:], sqc[ky][:], OP.add)
                            s0[kx + ky] = t
                    e.update(sqb=sqb, sqc=sqc, s0=s0)

                def st2(i):
                    """ACT rsqrt phase: rδ = AbsRsqrt(s0+1); δ = (s0+1)*rδ
                    (DVE STT, in place onto s0)."""
                    e = st[i]
                    rd, dl = {}, {}
                    for kk in ("mm", "mp", "pm", "pp"):
                        t = wtile("rd")
                        _act(t[:], e["s0"][kk][:], AF.Abs_reciprocal_sqrt,
                             bias=1.0)
                        rd[kk] = t
                    for kk in ("mm", "mp", "pm", "pp"):
                        t = e["s0"][kk]      # in place: (s0+1)*rδ -> δ
                        nc.vector.scalar_tensor_tensor(
                            t[:], t[:], 1.0, rd[kk][:], OP.add, OP.mult)
                        dl[kk] = t
                    e.update(rd=rd, dl=dl)

                def st3(i):
                    """pd sums, pair products, bc, targ = bc*rδ (DVE+Pool)."""
                    e = st[i]
                    bs, cs_, dl, rd = e["b"], e["c"], e["dl"], e["rd"]
                    cpd, bpd = {}, {}
                    pdi = 0
                    for kx in MP:
                        for ky in MP:
                            kk = kx + ky
                            t = wtile("cpd")
                            eng(ENG["pd"][pdi]).tensor_tensor(
                                t[:], cs_[ky][:], dl[kk][:], OP.add)
                            cpd[kk] = t
                            pdi += 1
                    for kx in MP:
                        for ky in MP:
                            kk = kx + ky
                            t = dl[kk]           # δ dead after bpd
                            eng(ENG["pd"][pdi]).tensor_tensor(
                                t[:], bs[kx][:], dl[kk][:], OP.add)
                            bpd[kk] = t
                            pdi += 1
                    # pair products; write onto one of the inputs
                    prod = {}
                    for kx in MP:      # b-side: fixed s1, product over s2
                        t = cpd[kx + "m"]
                        eng(ENG["prod"]).tensor_tensor(
                            t[:], cpd[kx + "m"][:], cpd[kx + "p"][:],
                            OP.mult)
                        prod["b" + kx] = t
                    for ky in MP:      # c-side: fixed s2, product over s1
                        t = bpd["m" + ky]
                        eng(ENG["prod"]).tensor_tensor(
                            t[:], bpd["m" + ky][:], bpd["p" + ky][:],
                            OP.mult)
                        prod["c" + ky] = t
                    # atan args: bc fresh, targ = bc*rδ in place on bc
                    targ = {}
                    for kx in MP:
                        for ky in MP:
                            kk = kx + ky
                            t = wtile("bc")
                            eng(ENG["bc"]).tensor_tensor(
                                t[:], bs[kx][:], cs_[ky][:], OP.mult)
                            eng(ENG["targ"]).tensor_tensor(
                                t[:], t[:], rd[kk][:], OP.mult)
                            targ[kk] = t
                    e.update(prod=prod, targ=targ)

                def st4(i):
                    """ACT ln phase: lax = Ln(sq+1) onto sq, lnp = Ln(prod)
                    onto prod."""
                    e = st[i]
                    lax, lnp = {}, {}
                    for sd, sq in (("b", e["sqb"]), ("c", e["sqc"])):
                        for k in MP:
                            t = sq[k]
                            _act(t[:], t[:], AF.Ln, bias=1.0)
                            lax[sd + k] = t
                    for sk, t in e["prod"].items():
                        _act(t[:], t[:], AF.Ln)
                        lnp[sk] = t
                    e.update(lax=lax, lnp=lnp)

                def st5(i):
                    """lnq = lax - lnp; blnq = b'*lnq; accumulate with
                    diag(-s_i)."""
                    e = st[i]
                    for sd, op in (("b", e["b"]), ("c", e["c"])):
                        for k in MP:
                            t = e["lax"][sd + k]
                            eng(ENG["sub"]).tensor_tensor(
                                t[:], t[:], e["lnp"][sd + k][:], OP.subtract)
                            eng(ENG["blnq"]).tensor_tensor(
                                t[:], op[k][:], t[:], OP.mult)
                            accum(t, i)

                def st6(i):
                    """ACT atan phase, in place on targ."""
                    e = st[i]
                    for kk, t in e["targ"].items():
                        _act(t[:], t[:], AF.Arctan)

                def st7(i):
                    """Accumulate the 4 atan tiles with diag(-s_i)."""
                    e = st[i]
                    for kk, t in e["targ"].items():
                        accum(t, i)
                    st[i] = {}   # drop tile refs

                stages = [st0, st1, st2, st3, st4, st5, st6, st7]
                for step, i in SCHEDULE:
                    stages[step](i)
                # evict PSUM -> SBUF (+endC) on ACT (Identity is in every
                # table -> no table load), then DMA out
                for h in range(W // HALF):
                    _act(res[:, h * HALF:(h + 1) * HALF], acc[h][:],
                         AF.Identity, bias=endC)
                nc.sync.dma_start(out[:, cs], res[:])
    nc.finalize()
    return nc


def _host_params(cx, cy, w, h, Pw, A, a, B_off, lx, ly, rows):
    """Per-core [128, NPAR] parameter matrix (per-partition scalars)."""
    pr = np.zeros((P, NPAR), dtype=np.float32)
    for i in range(NCHIP):
        la = a * lx[i]
        ha = a * ly[i]
        W0 = 0.5 * w[rows, i] / la
        H0 = 0.5 * h[rows, i] / ha
        cxl = cx[rows, i] / la
        cyl = cy[rows, i] / ha
        pr[:, 4 * i + 0] = np.repeat(W0 + cxl, REP)   # b'm = -x/(a lx) + .
        pr[:, 4 * i + 1] = np.repeat(W0 - cxl, REP)   # b'p = +x/(a lx) + .
        pr[:, 4 * i + 2] = np.repeat(H0 + cyl, REP)
        pr[:, 4 * i + 3] = np.repeat(H0 - cyl, REP)
    pr[:, 4 * NCHIP] = np.repeat(A * B_off * Pw[rows].sum(axis=1), REP)
    return np.ascontiguousarray(pr, dtype=np.float32)


def _host_diag(Pw, A, a, rows):
    """[128, NCHIP*128] fp32: per chiplet a diag(-Pi*A*a*2/sqrt(pi))."""
    dg = np.zeros((P, NCHIP * P), dtype=np.float32)
    idx = np.arange(P)
    for i in range(NCHIP):
        s = np.repeat(-C1 * A * a * Pw[rows, i], REP).astype(np.float32)
        dg[idx, i * P + idx] = s
    return np.ascontiguousarray(dg)


_CACHE = {}


def _get_executor(scal):
    """Build (once) the Bass program and a cached jitted SPMD callable.

    The callable takes the device-resident inputs as [8·128, ...] arrays
    sharded over 8 cores; the output scratch buffer is created on-device
    inside the same jit call (no host transfer)."""
    if "exec" in _CACHE:
        return _CACHE["exec"]

    import jax
    import jax.numpy as jnp
    from jax.sharding import Mesh, NamedSharding, PartitionSpec
    from jax.experimental.shard_map import shard_map
    from concourse import bass2jax
    import concourse.mybir as mybir

    nc = _build_program(scal)
    _CACHE["nc"] = nc
    bass2jax.install_neuronx_cc_hook()

    partition_name = (nc.partition_id_tensor.name
                      if nc.partition_id_tensor else None)
    in_names, out_names, out_avals = [], [], []
    for alloc in nc.m.functions[0].allocations:
        if not isinstance(alloc, mybir.MemoryLocationSet):
            continue
        name = alloc.memorylocations[0].name
        if alloc.kind == "ExternalInput":
            if name != partition_name:
                in_names.append(name)
        elif alloc.kind == "ExternalOutput":
            out_names.append(name)
            out_avals.append(jax.core.ShapedArray(
                tuple(alloc.tensor_shape), mybir.dt.np(alloc.dtype)))
    n_params = len(in_names)
    all_names = in_names + out_names
    if partition_name is not None:
        all_names = all_names + [partition_name]

    def _body(*args):
        # args = real inputs + one dummy buffer per output. On this
        # (axon/PJRT) path the output operands are inert: the NEFF rename
        # binds the bass "out" tensor to the custom-call RESULT buffer, so
        # the dummy is never read -- it only satisfies the hook's
        # param-order check. One persistent buffer is reused every call.
        operands = list(args)
        if partition_name is not None:
            operands.append(bass2jax.partition_id_tensor())
        outs = bass2jax._bass_exec_p.bind(
            *operands,
            out_avals=tuple(out_avals),
            in_names=tuple(all_names),
            out_names=tuple(out_names),
            lowering_input_output_aliases=(),
            sim_require_finite=True,
            sim_require_nnan=True,
            nc=nc,
        )
        return tuple(outs)

    devices = jax.devices()[:N_CORES]
    mesh = Mesh(np.asarray(devices), ("core",))
    sharding = NamedSharding(mesh, PartitionSpec("core"))
    sharded = jax.jit(
        shard_map(_body, mesh=mesh,
                  in_specs=(PartitionSpec("core"),) * (n_params + len(out_avals)),
                  out_specs=(PartitionSpec("core"),) * len(out_avals),
                  check_rep=False),
        keep_unused=True)

    # device-side dummy output operands (content never read)
    zshapes = [(N_CORES * s.shape[0], *s.shape[1:]) for s in out_avals]
    zdtypes = [s.dtype for s in out_avals]

    def _mk(shape_dtype):
        shape, dtype = shape_dtype
        return jax.jit(lambda: jnp.zeros(shape, dtype),
                       out_shardings=sharding)

    zeros_fns = [_mk(sd) for sd in zip(zshapes, zdtypes)]
    ex = {"sharded": sharded, "in_names": in_names, "zeros_fns": zeros_fns,
          "sharding": sharding, "n_params": n_params}
    _CACHE["exec"] = ex
    return ex


def _scal_from_inputs(a, lx, ly):
    af = float(np.asarray(a).reshape(-1)[0])
    lxf = np.asarray(lx, dtype=np.float64)
    lyf = np.asarray(ly, dtype=np.float64)
    return {
        "inv_la": [float(1.0 / (af * lxf[i])) for i in range(NCHIP)],
        "inv_ha": [float(1.0 / (af * lyf[i])) for i in range(NCHIP)],
    }


def _device_inputs(x, y, chiplets_x, chiplets_y, chiplets_width,
                   chiplets_height, chiplets_power, A, a, B_off, lx, ly):
    """Full-input -> per-core-stacked device arrays keyed by tensor name."""
    x = np.asarray(x, dtype=np.float32)
    y = np.asarray(y, dtype=np.float32)
    cx = np.asarray(chiplets_x, dtype=np.float32)
    cy = np.asarray(chiplets_y, dtype=np.float32)
    w = np.asarray(chiplets_width, dtype=np.float32)
    h = np.asarray(chiplets_height, dtype=np.float32)
    Pw = np.asarray(chiplets_power, dtype=np.float32)
    Af = float(np.asarray(A).reshape(-1)[0])
    af = float(np.asarray(a).reshape(-1)[0])
    Bf = float(np.asarray(B_off).reshape(-1)[0])
    lxf = np.asarray(lx, dtype=np.float64)
    lyf = np.asarray(ly, dtype=np.float64)

    xs = np.ascontiguousarray(x.reshape(N_CORES * P, F))
    ys = np.ascontiguousarray(y.reshape(N_CORES * P, F))
    prs = np.concatenate(
        [_host_params(cx, cy, w, h, Pw, Af, af, Bf, lxf, lyf,
                      slice(c * RPC, (c + 1) * RPC)) for c in range(N_CORES)],
        axis=0)
    dgs = np.concatenate(
        [_host_diag(Pw, Af, af, slice(c * RPC, (c + 1) * RPC))
         for c in range(N_CORES)], axis=0)
    return {"xin": xs, "yin": ys, "prm": prs, "dgf": dgs}


def run(x, y, chiplets_x, chiplets_y, chiplets_width, chiplets_height,
        chiplets_power, A, a, B_off, lx, ly, grid=None):
    import jax

    ex = _get_executor(_scal_from_inputs(a, lx, ly))
    arrs = _device_inputs(x, y, chiplets_x, chiplets_y, chiplets_width,
                          chiplets_height, chiplets_power, A, a, B_off,
                          lx, ly)
    ins = [jax.device_put(arrs[nm], ex["sharding"]) for nm in ex["in_names"]]
    scratch = [zf() for zf in ex["zeros_fns"]]
    out = ex["sharded"](*ins, *scratch)
    full = np.asarray(out[0]).reshape(B, G2).astype(np.float32, copy=False)
    return full


def kernel(**inputs):
    return run(**inputs)


# revision 28
# speedup vs baseline: 1.0204x; 1.0145x over previous
"""Trainium2 Bass kernel for the ChipletThermalModel problem.

Math per chiplet i, per grid point (summed over 16 chiplets), after
normalizing by `a` (F(a,b,c) = a*F(1, b/a, c/a), so a^2 -> 1 and the
overall factor a folds into the per-chiplet scale):
  b'± = pb± ± x/(a*lx),  c'± = pc± ± y/(a*ly)       (pb,pc host-precomputed)
  For the 4 sign combos k=(s1,s2):
    rδ_k = AbsRsqrt(s0_k + 1) = 1/δ_k   (ACT table op, ~4e-5 rel err)
    δ_k  = (s0_k + 1) * rδ_k            (one DVE STT)
  b-side (pair-merged over s2):
    lnq_b(s1) = ln(1+b'²) - ln((c'm+δ_{s1m})(c'p+δ_{s1p}))
    contribution -= s_i * b'_{s1} * lnq_b(s1)        (s_i = Pi*A*a*2/√π)
  c-side symmetric; atan side: contribution -= s_i * atan(b'c' * rδ_k)
  (the HW ISA has no tensor-tensor divide; recip-approx custom DVE ops
  measure 3.1us/op, AbsRsqrt on ACT is far cheaper) plus endC=A*B_off*ΣPi.

The -s_i scale is applied by the accumulating PE matmul with a diagonal
stationary matrix diag(-s_i[p]) (host-precomputed per partition), so no
elementwise scale ops remain.

Engine split per [128,1024] fp32 tile per chiplet, using MEASURED HW
throughputs (Pool mult 5.8us -> Pool gets only add/sub; DVE TT 1.20us,
DVE TS 1.00us, Pool add ~2.0us, ACT ~1.43us):
  ACT : 18 ops in 3 table phases (4 AbsRsqrt | 4+4 Ln | 4 Atan) + 2
        Square (in every table -> no extra load) + PSUM eviction.
  DVE : 4 prep tensor_scalar + 2 sq + 4 δ-STT + 16 mult TT.
  Pool: 16 add/sub TT.
  PE  : 16 matmul-halves [128,512] fp32 accumulating into PSUM.

Sharding: batch dim (64) split across 8 cores -> 8 rows/core, laid out as
[128 partitions, 4096]; per-(batch-row,chiplet) parameters become
per-partition [128,1] scalars and [128,128] diag blocks (host-precomputed).
"""
import sys
import numpy as np

for _p in ("/opt/trn_rl_repo",):
    if _p not in sys.path:
        sys.path.insert(0, _p)

N_CORES = 8
B, NCHIP, G2 = 64, 16, 65536
RPC = B // N_CORES            # batch rows per core = 8
P = 128                       # SBUF partitions
F = RPC * G2 // P             # free-dim columns per core = 4096
W = 1024                      # columns per processing group
NG = F // W                   # groups
WK_BUFS = 40                  # work-tile ring size
REP = P // RPC                # partitions per batch row = 16
NPAR = 4 * NCHIP + 1          # params columns (4 per chiplet + endC)
C1 = float(2.0 / np.sqrt(np.pi))

# Which of the flexible elementwise ops run on DVE ("d") vs Pool ("p") vs
# ACT ("a", unary-capable ops only). Lists give per-index picks.
# Measured HW: Pool can only afford adds/subs; all mults go to DVE/ACT.
ENG = {
    "sq": ["d", "d", "a", "a"],          # b'm², b'p², c'm², c'p²
    "s0": "p",
    "pd": ["d", "d", "p", "p", "p", "p", "p", "p"],
    "prod": "d", "sub": "p", "bc": "d", "targ": "d", "blnq": "d",
}


def _make_schedule(mode):
    """Emission order over (stage, chiplet). Engines dispatch in order with
    head-of-line blocking; pair-blocked ACT phases keep table loads at 3
    per chiplet pair, and hoisting the next pair's early stages keeps the
    vector engines fed while ACT chews a phase."""
    s = []
    if mode == "pairs":
        for pr in range(NCHIP // 2):
            i0, i1 = 2 * pr, 2 * pr + 1
            s += [(0, i0), (1, i0), (0, i1), (1, i1),
                  (2, i0), (2, i1), (3, i0), (3, i1),
                  (4, i0), (4, i1), (5, i0), (5, i1),
                  (6, i0), (6, i1), (7, i0), (7, i1)]
    elif mode == "sw":
        # software-pipelined: next pair's prep/squares emitted between this
        # pair's ln phase and blnq stage
        s += [(0, 0), (1, 0), (0, 1), (1, 1)]
        for pr in range(NCHIP // 2):
            i0, i1 = 2 * pr, 2 * pr + 1
            j0, j1 = i0 + 2, i1 + 2
            s += [(2, i0), (2, i1), (3, i0), (3, i1),
                  (4, i0), (4, i1),
                  (0, j0), (1, j0), (0, j1), (1, j1),
                  (5, i0), (5, i1),
                  (6, i0), (6, i1), (7, i0), (7, i1)]
    elif mode == "sw4":
        # deep rotation: every ACT phase consumes inputs produced a full
        # half-iteration earlier. Steady state works on pair p's tail
        # (ln/blnq/atan) while building pair q=p+1 from scratch.
        s += [(0, 0), (1, 0), (0, 1), (1, 1),
              (2, 0), (2, 1), (3, 0), (3, 1)]
        for pr in range(NCHIP // 2):
            i0, i1 = 2 * pr, 2 * pr + 1
            j0, j1 = i0 + 2, i1 + 2
            s += [(4, i0), (4, i1),
                  (0, j0), (1, j0), (0, j1), (1, j1),
                  (2, j0), (2, j1),
                  (3, j0), (3, j1),
                  (5, i0), (5, i1),
                  (6, i0), (7, i0), (6, i1), (7, i1)]
    return [t for t in s if 0 <= t[1] < NCHIP]


SCHEDULE = _make_schedule("sw4")


def _build_program(scal):
    """Build the Bass program. `scal` holds python-float per-chiplet scalars."""
    from concourse import bacc, tile
    import concourse.mybir as mybir
    import bass_rust as _bass_rust

    AF = mybir.ActivationFunctionType
    OP = mybir.AluOpType
    FP32 = mybir.dt.float32

    nc = bacc.Bacc("TRN2", target_bir_lowering=False, debug=False,
                   enable_asserts=False)

    # Pin the ACT instruction order with scheduler-only (nosync) dep edges:
    # the engine is in-order so same-engine ordering costs nothing at
    # runtime, but it stops the list scheduler from interleaving ops of
    # different table phases (table thrash).
    _act_prev = [None]

    def _act(out, in_, func, **kw):
        inst = nc.scalar.activation(out, in_, func, **kw)
        if _act_prev[0] is not None:
            _bass_rust.add_dep_helper(inst.ins, _act_prev[0], sync=False,
                                      reason="act table phase order")
        _act_prev[0] = inst.ins
        return inst

    FP16 = mybir.dt.float16

    xin = nc.dram_tensor("xin", [P, F], FP32, kind="ExternalInput")
    yin = nc.dram_tensor("yin", [P, F], FP32, kind="ExternalInput")
    prm = nc.dram_tensor("prm", [P, NPAR], FP32, kind="ExternalInput")
    dgf = nc.dram_tensor("dgf", [P, NCHIP * P], FP32, kind="ExternalInput")
    dgh = nc.dram_tensor("dgh", [P, NCHIP * P], FP16, kind="ExternalInput")
    out = nc.dram_tensor("out", [P, F], FP32, kind="ExternalOutput")

    inv_la = scal["inv_la"]
    inv_ha = scal["inv_ha"]

    MP = ("m", "p")
    HALF = W // 2

    def eng(which):
        return nc.vector if which == "d" else nc.gpsimd

    with tile.TileContext(nc) as tc:
        with tc.tile_pool(name="cst", bufs=1) as cst, \
             tc.tile_pool(name="io", bufs=2) as io, \
             tc.tile_pool(name="ps", bufs=2, space="PSUM") as ps, \
             tc.tile_pool(name="wk16", bufs=4) as wk16, \
             tc.tile_pool(name="wk", bufs=WK_BUFS) as wk:
            prmt = cst.tile([P, NPAR], FP32)
            nc.sync.dma_start(prmt[:], prm[:])
            dgt = cst.tile([P, NCHIP * P], FP32)
            nc.sync.dma_start(dgt[:], dgf[:])
            dgth = cst.tile([P, NCHIP * P], FP16)
            nc.sync.dma_start(dgth[:], dgh[:])

            def pcol(i, k):           # [128,1] per-partition param AP
                return prmt[:, 4 * i + k: 4 * i + k + 1]

            endC = prmt[:, 4 * NCHIP: 4 * NCHIP + 1]

            for g in range(NG):
                cs = slice(g * W, (g + 1) * W)
                xt = io.tile([P, W], FP32, tag="xt")
                yt = io.tile([P, W], FP32, tag="yt")
                res = io.tile([P, W], FP32, tag="res")
                nc.sync.dma_start(xt[:], xin[:, cs])
                nc.sync.dma_start(yt[:], yin[:, cs])
                # PSUM accumulators: one bank per half-tile (matmul moving
                # free dim is capped at 512)
                acc = [ps.tile([P, HALF], FP32, tag=f"acc{h}",
                               name=f"acc{h}")
                       for h in range(W // HALF)]
                mm_count = [0]
                MM_TOTAL = NCHIP * 8 * (W // HALF)

                def accum(t, i, half=False):
                    """res_psum += diag(-s_i) @ t on the PE; fp16 moving
                    tiles (atan outputs) pair with the fp16 diag at ~4x."""
                    stat = (dgth if half else dgt)[:, i * P:(i + 1) * P]
                    for h, a_ in enumerate(acc):
                        first = mm_count[0] < len(acc)
                        last = mm_count[0] >= MM_TOTAL - len(acc)
                        nc.tensor.matmul(
                            a_[:], stat, t[:, h * HALF:(h + 1) * HALF],
                            start=first, stop=last)
                        mm_count[0] += 1

                def wtile(nm):
                    return wk.tile([P, W], FP32, tag="wk", name=nm)

                # per-chiplet state dicts, keyed by chiplet index
                st = [dict() for _ in range(NCHIP)]

                def st0(i):
                    """b'±, c'± via DVE tensor_scalar (x*(±1/(a*l)) + p)."""
                    e = st[i]
                    bs, cs_ = {}, {}
                    for k, sgn, col in (("m", -1.0, 0), ("p", 1.0, 1)):
                        t = wtile("b" + k)
                        nc.vector.tensor_scalar(t[:], xt[:], sgn * inv_la[i],
                                                pcol(i, col), OP.mult, OP.add)
                        bs[k] = t
                    for k, sgn, col in (("m", -1.0, 2), ("p", 1.0, 3)):
                        t = wtile("c" + k)
                        nc.vector.tensor_scalar(t[:], yt[:], sgn * inv_ha[i],
                                                pcol(i, col), OP.mult, OP.add)
                        cs_[k] = t
                    e.update(b=bs, c=cs_)

                def st1(i):
                    """squares and s0 = b'² + c'² per combo."""
                    e = st[i]
                    sqb, sqc = {}, {}
                    srcs = [("b", e["b"]), ("c", e["c"])]
                    sqi = 0
                    for nm, src in srcs:
                        d = sqb if nm == "b" else sqc
                        for k in MP:
                            t = wtile("sq" + nm)
                            which = ENG["sq"][sqi]
                            sqi += 1
                            if which == "a":
                                _act(t[:], src[k][:], AF.Square)
                            else:
                                eng(which).tensor_tensor(
                                    t[:], src[k][:], src[k][:], OP.mult)
                            d[k] = t
                    s0 = {}
                    for kx in MP:
                        for ky in MP:
                            t = wtile("s0")
                            eng(ENG["s0"]).tensor_tensor(
                                t[:], sqb[kx][:], sqc[ky][:], OP.add)
                            s0[kx + ky] = t
                    e.update(sqb=sqb, sqc=sqc, s0=s0)

                def st2(i):
                    """ACT rsqrt phase: rδ = AbsRsqrt(s0+1); δ = (s0+1)*rδ
                    (DVE STT, in place onto s0)."""
                    e = st[i]
                    rd, dl = {}, {}
                    for kk in ("mm", "mp", "pm", "pp"):
                        t = wtile("rd")
                        _act(t[:], e["s0"][kk][:], AF.Abs_reciprocal_sqrt,
                             bias=1.0)
                        rd[kk] = t
                    for kk in ("mm", "mp", "pm", "pp"):
                        t = e["s0"][kk]      # in place: (s0+1)*rδ -> δ
                        nc.vector.scalar_tensor_tensor(
                            t[:], t[:], 1.0, rd[kk][:], OP.add, OP.mult)
                        dl[kk] = t
                    e.update(rd=rd, dl=dl)

                def st3(i):
                    """pd sums, pair products, bc, targ = bc*rδ (DVE+Pool)."""
                    e = st[i]
                    bs, cs_, dl, rd = e["b"], e["c"], e["dl"], e["rd"]
                    cpd, bpd = {}, {}
                    pdi = 0
                    for kx in MP:
                        for ky in MP:
                            kk = kx + ky
                            t = wtile("cpd")
                            eng(ENG["pd"][pdi]).tensor_tensor(
                                t[:], cs_[ky][:], dl[kk][:], OP.add)
                            cpd[kk] = t
                            pdi += 1
                    for kx in MP:
                        for ky in MP:
                            kk = kx + ky
                            t = dl[kk]           # δ dead after bpd
                            eng(ENG["pd"][pdi]).tensor_tensor(
                                t[:], bs[kx][:], dl[kk][:], OP.add)
                            bpd[kk] = t
                            pdi += 1
                    # pair products; write onto one of the inputs
                    prod = {}
                    for kx in MP:      # b-side: fixed s1, product over s2
                        t = cpd[kx + "m"]
                        eng(ENG["prod"]).tensor_tensor(
                            t[:], cpd[kx + "m"][:], cpd[kx + "p"][:],
                            OP.mult)
                        prod["b" + kx] = t
                    for ky in MP:      # c-side: fixed s2, product over s1
                        t = bpd["m" + ky]
                        eng(ENG["prod"]).tensor_tensor(
                            t[:], bpd["m" + ky][:], bpd["p" + ky][:],
                            OP.mult)
                        prod["c" + ky] = t
                    # atan args: bc fresh, targ = bc*rδ in place on bc
                    targ = {}
                    for kx in MP:
                        for ky in MP:
                            kk = kx + ky
                            t = wtile("bc")
                            eng(ENG["bc"]).tensor_tensor(
                                t[:], bs[kx][:], cs_[ky][:], OP.mult)
                            eng(ENG["targ"]).tensor_tensor(
                                t[:], t[:], rd[kk][:], OP.mult)
                            targ[kk] = t
                    e.update(prod=prod, targ=targ)

                def st4(i):
                    """ACT ln phase: lax = Ln(sq+1) onto sq, lnp = Ln(prod)
                    onto prod."""
                    e = st[i]
                    lax, lnp = {}, {}
                    for sd, sq in (("b", e["sqb"]), ("c", e["sqc"])):
                        for k in MP:
                            t = sq[k]
                            _act(t[:], t[:], AF.Ln, bias=1.0)
                            lax[sd + k] = t
                    for sk, t in e["prod"].items():
                        _act(t[:], t[:], AF.Ln)
                        lnp[sk] = t
                    e.update(lax=lax, lnp=lnp)

                def st5(i):
                    """lnq = lax - lnp; blnq = b'*lnq; accumulate with
                    diag(-s_i)."""
                    e = st[i]
                    for sd, op in (("b", e["b"]), ("c", e["c"])):
                        for k in MP:
                            t = e["lax"][sd + k]
                            eng(ENG["sub"]).tensor_tensor(
                                t[:], t[:], e["lnp"][sd + k][:], OP.subtract)
                            eng(ENG["blnq"]).tensor_tensor(
                                t[:], op[k][:], t[:], OP.mult)
                            accum(t, i)

                def st6(i):
                    """ACT atan phase, fp16 outputs (|at| <= 1.4 so fp16
                    adds < 1e-3 absolute; the PE then accumulates them at
                    ~4x the fp32 matmul rate)."""
                    e = st[i]
                    at = {}
                    for kk, t in e["targ"].items():
                        th = wk16.tile([P, W], FP16, tag="at16", name="at16")
                        _act(th[:], t[:], AF.Arctan)
                        at[kk] = th
                    e.update(at=at)

                def st7(i):
                    """Accumulate the 4 atan tiles with fp16 diag(-s_i)."""
                    e = st[i]
                    for kk, t in e["at"].items():
                        accum(t, i, half=True)
                    st[i] = {}   # drop tile refs

                stages = [st0, st1, st2, st3, st4, st5, st6, st7]
                for step, i in SCHEDULE:
                    stages[step](i)
                # evict PSUM -> SBUF (+endC) on ACT (Identity is in every
                # table -> no table load), then DMA out
                for h in range(W // HALF):
                    _act(res[:, h * HALF:(h + 1) * HALF], acc[h][:],
                         AF.Identity, bias=endC)
                nc.sync.dma_start(out[:, cs], res[:])
    nc.finalize()
    return nc


def _host_params(cx, cy, w, h, Pw, A, a, B_off, lx, ly, rows):
    """Per-core [128, NPAR] parameter matrix (per-partition scalars)."""
    pr = np.zeros((P, NPAR), dtype=np.float32)
    for i in range(NCHIP):
        la = a * lx[i]
        ha = a * ly[i]
        W0 = 0.5 * w[rows, i] / la
        H0 = 0.5 * h[rows, i] / ha
        cxl = cx[rows, i] / la
        cyl = cy[rows, i] / ha
        pr[:, 4 * i + 0] = np.repeat(W0 + cxl, REP)   # b'm = -x/(a lx) + .
        pr[:, 4 * i + 1] = np.repeat(W0 - cxl, REP)   # b'p = +x/(a lx) + .
        pr[:, 4 * i + 2] = np.repeat(H0 + cyl, REP)
        pr[:, 4 * i + 3] = np.repeat(H0 - cyl, REP)
    pr[:, 4 * NCHIP] = np.repeat(A * B_off * Pw[rows].sum(axis=1), REP)
    return np.ascontiguousarray(pr, dtype=np.float32)


def _host_diag(Pw, A, a, rows):
    """[128, NCHIP*128] fp32: per chiplet a diag(-Pi*A*a*2/sqrt(pi))."""
    dg = np.zeros((P, NCHIP * P), dtype=np.float32)
    idx = np.arange(P)
    for i in range(NCHIP):
        s = np.repeat(-C1 * A * a * Pw[rows, i], REP).astype(np.float32)
        dg[idx, i * P + idx] = s
    return np.ascontiguousarray(dg)


_CACHE = {}


def _get_executor(scal):
    """Build (once) the Bass program and a cached jitted SPMD callable.

    The callable takes the device-resident inputs as [8·128, ...] arrays
    sharded over 8 cores; the output scratch buffer is created on-device
    inside the same jit call (no host transfer)."""
    if "exec" in _CACHE:
        return _CACHE["exec"]

    import jax
    import jax.numpy as jnp
    from jax.sharding import Mesh, NamedSharding, PartitionSpec
    from jax.experimental.shard_map import shard_map
    from concourse import bass2jax
    import concourse.mybir as mybir

    nc = _build_program(scal)
    _CACHE["nc"] = nc
    bass2jax.install_neuronx_cc_hook()

    partition_name = (nc.partition_id_tensor.name
                      if nc.partition_id_tensor else None)
    in_names, out_names, out_avals = [], [], []
    for alloc in nc.m.functions[0].allocations:
        if not isinstance(alloc, mybir.MemoryLocationSet):
            continue
        name = alloc.memorylocations[0].name
        if alloc.kind == "ExternalInput":
            if name != partition_name:
                in_names.append(name)
        elif alloc.kind == "ExternalOutput":
            out_names.append(name)
            out_avals.append(jax.core.ShapedArray(
                tuple(alloc.tensor_shape), mybir.dt.np(alloc.dtype)))
    n_params = len(in_names)
    all_names = in_names + out_names
    if partition_name is not None:
        all_names = all_names + [partition_name]

    def _body(*args):
        # args = real inputs + one dummy buffer per output. On this
        # (axon/PJRT) path the output operands are inert: the NEFF rename
        # binds the bass "out" tensor to the custom-call RESULT buffer, so
        # the dummy is never read -- it only satisfies the hook's
        # param-order check. One persistent buffer is reused every call.
        operands = list(args)
        if partition_name is not None:
            operands.append(bass2jax.partition_id_tensor())
        outs = bass2jax._bass_exec_p.bind(
            *operands,
            out_avals=tuple(out_avals),
            in_names=tuple(all_names),
            out_names=tuple(out_names),
            lowering_input_output_aliases=(),
            sim_require_finite=True,
            sim_require_nnan=True,
            nc=nc,
        )
        return tuple(outs)

    devices = jax.devices()[:N_CORES]
    mesh = Mesh(np.asarray(devices), ("core",))
    sharding = NamedSharding(mesh, PartitionSpec("core"))
    sharded = jax.jit(
        shard_map(_body, mesh=mesh,
                  in_specs=(PartitionSpec("core"),) * (n_params + len(out_avals)),
                  out_specs=(PartitionSpec("core"),) * len(out_avals),
                  check_rep=False),
        keep_unused=True)

    # device-side dummy output operands (content never read)
    zshapes = [(N_CORES * s.shape[0], *s.shape[1:]) for s in out_avals]
    zdtypes = [s.dtype for s in out_avals]

    def _mk(shape_dtype):
        shape, dtype = shape_dtype
        return jax.jit(lambda: jnp.zeros(shape, dtype),
                       out_shardings=sharding)

    zeros_fns = [_mk(sd) for sd in zip(zshapes, zdtypes)]
    ex = {"sharded": sharded, "in_names": in_names, "zeros_fns": zeros_fns,
          "sharding": sharding, "n_params": n_params}
    _CACHE["exec"] = ex
    return ex


def _scal_from_inputs(a, lx, ly):
    af = float(np.asarray(a).reshape(-1)[0])
    lxf = np.asarray(lx, dtype=np.float64)
    lyf = np.asarray(ly, dtype=np.float64)
    return {
        "inv_la": [float(1.0 / (af * lxf[i])) for i in range(NCHIP)],
        "inv_ha": [float(1.0 / (af * lyf[i])) for i in range(NCHIP)],
    }


def _device_inputs(x, y, chiplets_x, chiplets_y, chiplets_width,
                   chiplets_height, chiplets_power, A, a, B_off, lx, ly):
    """Full-input -> per-core-stacked device arrays keyed by tensor name."""
    x = np.asarray(x, dtype=np.float32)
    y = np.asarray(y, dtype=np.float32)
    cx = np.asarray(chiplets_x, dtype=np.float32)
    cy = np.asarray(chiplets_y, dtype=np.float32)
    w = np.asarray(chiplets_width, dtype=np.float32)
    h = np.asarray(chiplets_height, dtype=np.float32)
    Pw = np.asarray(chiplets_power, dtype=np.float32)
    Af = float(np.asarray(A).reshape(-1)[0])
    af = float(np.asarray(a).reshape(-1)[0])
    Bf = float(np.asarray(B_off).reshape(-1)[0])
    lxf = np.asarray(lx, dtype=np.float64)
    lyf = np.asarray(ly, dtype=np.float64)

    xs = np.ascontiguousarray(x.reshape(N_CORES * P, F))
    ys = np.ascontiguousarray(y.reshape(N_CORES * P, F))
    prs = np.concatenate(
        [_host_params(cx, cy, w, h, Pw, Af, af, Bf, lxf, lyf,
                      slice(c * RPC, (c + 1) * RPC)) for c in range(N_CORES)],
        axis=0)
    dgs = np.concatenate(
        [_host_diag(Pw, Af, af, slice(c * RPC, (c + 1) * RPC))
         for c in range(N_CORES)], axis=0)
    return {"xin": xs, "yin": ys, "prm": prs, "dgf": dgs,
            "dgh": dgs.astype(np.float16)}


def run(x, y, chiplets_x, chiplets_y, chiplets_width, chiplets_height,
        chiplets_power, A, a, B_off, lx, ly, grid=None):
    import jax

    ex = _get_executor(_scal_from_inputs(a, lx, ly))
    arrs = _device_inputs(x, y, chiplets_x, chiplets_y, chiplets_width,
                          chiplets_height, chiplets_power, A, a, B_off,
                          lx, ly)
    ins = [jax.device_put(arrs[nm], ex["sharding"]) for nm in ex["in_names"]]
    scratch = [zf() for zf in ex["zeros_fns"]]
    out = ex["sharded"](*ins, *scratch)
    full = np.asarray(out[0]).reshape(B, G2).astype(np.float32, copy=False)
    return full


def kernel(**inputs):
    return run(**inputs)


# revision 29
# speedup vs baseline: 1.0345x; 1.0138x over previous
"""Trainium2 Bass kernel for the ChipletThermalModel problem.

Math per chiplet i, per grid point (summed over 16 chiplets), after
normalizing by `a` (F(a,b,c) = a*F(1, b/a, c/a), so a^2 -> 1 and the
overall factor a folds into the per-chiplet scale):
  b'± = pb± ± x/(a*lx),  c'± = pc± ± y/(a*ly)       (pb,pc host-precomputed)
  For the 4 sign combos k=(s1,s2):
    rδ_k = AbsRsqrt(s0_k + 1) = 1/δ_k   (ACT table op, ~4e-5 rel err)
    δ_k  = (s0_k + 1) * rδ_k            (one DVE STT)
  b-side (pair-merged over s2):
    lnq_b(s1) = ln(1+b'²) - ln((c'm+δ_{s1m})(c'p+δ_{s1p}))
    contribution -= s_i * b'_{s1} * lnq_b(s1)        (s_i = Pi*A*a*2/√π)
  c-side symmetric; atan side: contribution -= s_i * atan(b'c' * rδ_k)
  (the HW ISA has no tensor-tensor divide; recip-approx custom DVE ops
  measure 3.1us/op, AbsRsqrt on ACT is far cheaper) plus endC=A*B_off*ΣPi.

The -s_i scale is applied by the accumulating PE matmul with a diagonal
stationary matrix diag(-s_i[p]) (host-precomputed per partition), so no
elementwise scale ops remain.

Engine split per [128,1024] fp32 tile per chiplet, using MEASURED HW
throughputs (Pool mult 5.8us -> Pool gets only add/sub; DVE TT 1.20us,
DVE TS 1.00us, Pool add ~2.0us, ACT ~1.43us):
  ACT : 18 ops in 3 table phases (4 AbsRsqrt | 4+4 Ln | 4 Atan) + 2
        Square (in every table -> no extra load) + PSUM eviction.
  DVE : 4 prep tensor_scalar + 2 sq + 4 δ-STT + 16 mult TT.
  Pool: 16 add/sub TT.
  PE  : 16 matmul-halves [128,512] fp32 accumulating into PSUM.

Sharding: batch dim (64) split across 8 cores -> 8 rows/core, laid out as
[128 partitions, 4096]; per-(batch-row,chiplet) parameters become
per-partition [128,1] scalars and [128,128] diag blocks (host-precomputed).
"""
import sys
import numpy as np

for _p in ("/opt/trn_rl_repo",):
    if _p not in sys.path:
        sys.path.insert(0, _p)

N_CORES = 8
B, NCHIP, G2 = 64, 16, 65536
RPC = B // N_CORES            # batch rows per core = 8
P = 128                       # SBUF partitions
F = RPC * G2 // P             # free-dim columns per core = 4096
W = 1024                      # columns per processing group
NG = F // W                   # groups
WK_BUFS = 40                  # work-tile ring size
REP = P // RPC                # partitions per batch row = 16
NPAR = 4 * NCHIP + 1          # params columns (4 per chiplet + endC)
C1 = float(2.0 / np.sqrt(np.pi))

# Which of the flexible elementwise ops run on DVE ("d") vs Pool ("p") vs
# ACT ("a", unary-capable ops only). Lists give per-index picks.
# Measured HW: Pool can only afford adds/subs; all mults go to DVE/ACT.
ENG = {
    "sq": ["d", "d", "a", "a"],          # b'm², b'p², c'm², c'p²
    "s0": "p",
    "pd": ["d", "d", "p", "p", "p", "p", "p", "p"],
    "prod": "d", "sub": "p", "bc": "d", "targ": "d", "blnq": "d",
}


def _make_schedule(mode):
    """Emission order over (stage, chiplet). Engines dispatch in order with
    head-of-line blocking; pair-blocked ACT phases keep table loads at 3
    per chiplet pair, and hoisting the next pair's early stages keeps the
    vector engines fed while ACT chews a phase."""
    s = []
    if mode == "pairs":
        for pr in range(NCHIP // 2):
            i0, i1 = 2 * pr, 2 * pr + 1
            s += [(0, i0), (1, i0), (0, i1), (1, i1),
                  (2, i0), (2, i1), (3, i0), (3, i1),
                  (4, i0), (4, i1), (5, i0), (5, i1),
                  (6, i0), (6, i1), (7, i0), (7, i1)]
    elif mode == "sw":
        # software-pipelined: next pair's prep/squares emitted between this
        # pair's ln phase and blnq stage
        s += [(0, 0), (1, 0), (0, 1), (1, 1)]
        for pr in range(NCHIP // 2):
            i0, i1 = 2 * pr, 2 * pr + 1
            j0, j1 = i0 + 2, i1 + 2
            s += [(2, i0), (2, i1), (3, i0), (3, i1),
                  (4, i0), (4, i1),
                  (0, j0), (1, j0), (0, j1), (1, j1),
                  (5, i0), (5, i1),
                  (6, i0), (6, i1), (7, i0), (7, i1)]
    elif mode == "sw4":
        # deep rotation: every ACT phase consumes inputs produced a full
        # half-iteration earlier. Steady state works on pair p's tail
        # (ln/blnq/atan) while building pair q=p+1 from scratch.
        s += [(0, 0), (1, 0), (0, 1), (1, 1),
              (2, 0), (2, 1), (3, 0), (3, 1)]
        for pr in range(NCHIP // 2):
            i0, i1 = 2 * pr, 2 * pr + 1
            j0, j1 = i0 + 2, i1 + 2
            s += [(4, i0), (4, i1),
                  (0, j0), (1, j0), (0, j1), (1, j1),
                  (2, j0), (2, j1),
                  (3, j0), (3, j1),
                  (5, i0), (5, i1),
                  (6, i0), (7, i0), (6, i1), (7, i1)]
    return [t for t in s if 0 <= t[1] < NCHIP]


SCHEDULE = _make_schedule("sw4")


def _build_program(scal):
    """Build the Bass program. `scal` holds python-float per-chiplet scalars."""
    from concourse import bacc, tile
    import concourse.mybir as mybir
    import bass_rust as _bass_rust

    AF = mybir.ActivationFunctionType
    OP = mybir.AluOpType
    FP32 = mybir.dt.float32

    nc = bacc.Bacc("TRN2", target_bir_lowering=False, debug=False,
                   enable_asserts=False)

    # Pin the ACT instruction order with scheduler-only (nosync) dep edges:
    # the engine is in-order so same-engine ordering costs nothing at
    # runtime, but it stops the list scheduler from interleaving ops of
    # different table phases (table thrash).
    _act_prev = [None]

    def _act(out, in_, func, **kw):
        inst = nc.scalar.activation(out, in_, func, **kw)
        if _act_prev[0] is not None:
            _bass_rust.add_dep_helper(inst.ins, _act_prev[0], sync=False,
                                      reason="act table phase order")
        _act_prev[0] = inst.ins
        return inst

    FP16 = mybir.dt.float16

    xin = nc.dram_tensor("xin", [P, F], FP32, kind="ExternalInput")
    yin = nc.dram_tensor("yin", [P, F], FP32, kind="ExternalInput")
    prm = nc.dram_tensor("prm", [P, NPAR], FP32, kind="ExternalInput")
    dgf = nc.dram_tensor("dgf", [P, NCHIP * P], FP32, kind="ExternalInput")
    dgh = nc.dram_tensor("dgh", [P, NCHIP * P], FP16, kind="ExternalInput")
    out = nc.dram_tensor("out", [P, F], FP32, kind="ExternalOutput")

    inv_la = scal["inv_la"]
    inv_ha = scal["inv_ha"]

    MP = ("m", "p")
    HALF = W // 2

    def eng(which):
        return nc.vector if which == "d" else nc.gpsimd

    with tile.TileContext(nc) as tc:
        with tc.tile_pool(name="cst", bufs=1) as cst, \
             tc.tile_pool(name="io", bufs=2) as io, \
             tc.tile_pool(name="ps", bufs=2, space="PSUM") as ps, \
             tc.tile_pool(name="wk16", bufs=4) as wk16, \
             tc.tile_pool(name="wk", bufs=WK_BUFS) as wk:
            prmt = cst.tile([P, NPAR], FP32)
            nc.sync.dma_start(prmt[:], prm[:])
            dgt = cst.tile([P, NCHIP * P], FP32)
            nc.sync.dma_start(dgt[:], dgf[:])
            dgth = cst.tile([P, NCHIP * P], FP16)
            nc.sync.dma_start(dgth[:], dgh[:])

            def pcol(i, k):           # [128,1] per-partition param AP
                return prmt[:, 4 * i + k: 4 * i + k + 1]

            endC = prmt[:, 4 * NCHIP: 4 * NCHIP + 1]

            for g in range(NG):
                cs = slice(g * W, (g + 1) * W)
                xt = io.tile([P, W], FP32, tag="xt")
                yt = io.tile([P, W], FP32, tag="yt")
                res = io.tile([P, W], FP32, tag="res")
                nc.sync.dma_start(xt[:], xin[:, cs])
                nc.sync.dma_start(yt[:], yin[:, cs])
                # PSUM accumulators: one bank per half-tile (matmul moving
                # free dim is capped at 512)
                acc = [ps.tile([P, HALF], FP32, tag=f"acc{h}",
                               name=f"acc{h}")
                       for h in range(W // HALF)]
                mm_count = [0]
                MM_TOTAL = NCHIP * 8 * (W // HALF)

                def accum(t, i, half=False):
                    """res_psum += diag(-s_i) @ t on the PE; fp16 moving
                    tiles (atan outputs) pair with the fp16 diag at ~4x."""
                    stat = (dgth if half else dgt)[:, i * P:(i + 1) * P]
                    for h, a_ in enumerate(acc):
                        first = mm_count[0] < len(acc)
                        last = mm_count[0] >= MM_TOTAL - len(acc)
                        nc.tensor.matmul(
                            a_[:], stat, t[:, h * HALF:(h + 1) * HALF],
                            start=first, stop=last)
                        mm_count[0] += 1

                def wtile(nm):
                    return wk.tile([P, W], FP32, tag="wk", name=nm)

                # per-chiplet state dicts, keyed by chiplet index
                st = [dict() for _ in range(NCHIP)]

                def st0(i):
                    """b'±, c'± via DVE tensor_scalar (x*(±1/(a*l)) + p)."""
                    e = st[i]
                    bs, cs_ = {}, {}
                    for k, sgn, col in (("m", -1.0, 0), ("p", 1.0, 1)):
                        t = wtile("b" + k)
                        nc.vector.tensor_scalar(t[:], xt[:], sgn * inv_la[i],
                                                pcol(i, col), OP.mult, OP.add)
                        bs[k] = t
                    for k, sgn, col in (("m", -1.0, 2), ("p", 1.0, 3)):
                        t = wtile("c" + k)
                        nc.vector.tensor_scalar(t[:], yt[:], sgn * inv_ha[i],
                                                pcol(i, col), OP.mult, OP.add)
                        cs_[k] = t
                    e.update(b=bs, c=cs_)

                def st1(i):
                    """squares and s0 = b'² + c'² per combo."""
                    e = st[i]
                    sqb, sqc = {}, {}
                    srcs = [("b", e["b"]), ("c", e["c"])]
                    sqi = 0
                    for nm, src in srcs:
                        d = sqb if nm == "b" else sqc
                        for k in MP:
                            t = wtile("sq" + nm)
                            which = ENG["sq"][sqi]
                            sqi += 1
                            if which == "a":
                                _act(t[:], src[k][:], AF.Square)
                            else:
                                eng(which).tensor_tensor(
                                    t[:], src[k][:], src[k][:], OP.mult)
                            d[k] = t
                    s0 = {}
                    for kx in MP:
                        for ky in MP:
                            t = wtile("s0")
                            eng(ENG["s0"]).tensor_tensor(
                                t[:], sqb[kx][:], sqc[ky][:], OP.add)
                            s0[kx + ky] = t
                    e.update(sqb=sqb, sqc=sqc, s0=s0)

                def st2(i):
                    """ACT rsqrt phase: rδ = AbsRsqrt(s0+1); δ = (s0+1)*rδ
                    (DVE STT, in place onto s0)."""
                    e = st[i]
                    rd, dl = {}, {}
                    for kk in ("mm", "mp", "pm", "pp"):
                        t = wtile("rd")
                        _act(t[:], e["s0"][kk][:], AF.Abs_reciprocal_sqrt,
                             bias=1.0)
                        rd[kk] = t
                    for kk in ("mm", "mp", "pm", "pp"):
                        t = e["s0"][kk]      # in place: (s0+1)*rδ -> δ
                        nc.vector.scalar_tensor_tensor(
                            t[:], t[:], 1.0, rd[kk][:], OP.add, OP.mult)
                        dl[kk] = t
                    e.update(rd=rd, dl=dl)

                def st3(i):
                    """pd sums, pair products, bc, targ = bc*rδ (DVE+Pool)."""
                    e = st[i]
                    bs, cs_, dl, rd = e["b"], e["c"], e["dl"], e["rd"]
                    cpd, bpd = {}, {}
                    pdi = 0
                    for kx in MP:
                        for ky in MP:
                            kk = kx + ky
                            t = wtile("cpd")
                            eng(ENG["pd"][pdi]).tensor_tensor(
                                t[:], cs_[ky][:], dl[kk][:], OP.add)
                            cpd[kk] = t
                            pdi += 1
                    for kx in MP:
                        for ky in MP:
                            kk = kx + ky
                            t = dl[kk]           # δ dead after bpd
                            eng(ENG["pd"][pdi]).tensor_tensor(
                                t[:], bs[kx][:], dl[kk][:], OP.add)
                            bpd[kk] = t
                            pdi += 1
                    # pair products; write onto one of the inputs
                    prod = {}
                    for kx in MP:      # b-side: fixed s1, product over s2
                        t = cpd[kx + "m"]
                        eng(ENG["prod"]).tensor_tensor(
                            t[:], cpd[kx + "m"][:], cpd[kx + "p"][:],
                            OP.mult)
                        prod["b" + kx] = t
                    for ky in MP:      # c-side: fixed s2, product over s1
                        t = bpd["m" + ky]
                        eng(ENG["prod"]).tensor_tensor(
                            t[:], bpd["m" + ky][:], bpd["p" + ky][:],
                            OP.mult)
                        prod["c" + ky] = t
                    # atan args: bc fresh, targ = bc*rδ in place on bc
                    targ = {}
                    for kx in MP:
                        for ky in MP:
                            kk = kx + ky
                            t = wtile("bc")
                            eng(ENG["bc"]).tensor_tensor(
                                t[:], bs[kx][:], cs_[ky][:], OP.mult)
                            eng(ENG["targ"]).tensor_tensor(
                                t[:], t[:], rd[kk][:], OP.mult)
                            targ[kk] = t
                    e.update(prod=prod, targ=targ)

                def st4(i):
                    """ACT ln phase: lax = Ln(sq+1) onto sq, lnp = Ln(prod)
                    onto prod."""
                    e = st[i]
                    lax, lnp = {}, {}
                    for sd, sq in (("b", e["sqb"]), ("c", e["sqc"])):
                        for k in MP:
                            t = sq[k]
                            _act(t[:], t[:], AF.Ln, bias=1.0)
                            lax[sd + k] = t
                    for sk, t in e["prod"].items():
                        _act(t[:], t[:], AF.Ln)
                        lnp[sk] = t
                    e.update(lax=lax, lnp=lnp)

                def st5(i):
                    """lnq = lax - lnp; blnq = b'*lnq; accumulate with
                    diag(-s_i)."""
                    e = st[i]
                    for sd, op in (("b", e["b"]), ("c", e["c"])):
                        for k in MP:
                            t = e["lax"][sd + k]
                            eng(ENG["sub"]).tensor_tensor(
                                t[:], t[:], e["lnp"][sd + k][:], OP.subtract)
                            eng(ENG["blnq"]).tensor_tensor(
                                t[:], op[k][:], t[:], OP.mult)
                            accum(t, i)

                def st6(i):
                    """ACT atan phase, fp16 outputs (|at| <= 1.4 so fp16
                    adds < 1e-3 absolute; the PE then accumulates them at
                    ~4x the fp32 matmul rate)."""
                    e = st[i]
                    at = {}
                    for kk, t in e["targ"].items():
                        th = wk16.tile([P, W], FP16, tag="at16", name="at16")
                        _act(th[:], t[:], AF.Arctan)
                        at[kk] = th
                    e.update(at=at)

                def st7(i):
                    """Accumulate the 4 atan tiles with fp16 diag(-s_i)."""
                    e = st[i]
                    for kk, t in e["at"].items():
                        accum(t, i, half=True)
                    st[i] = {}   # drop tile refs

                stages = [st0, st1, st2, st3, st4, st5, st6, st7]
                for step, i in SCHEDULE:
                    stages[step](i)
                # evict PSUM -> SBUF (+endC) on ACT (Identity is in every
                # table -> no table load). Deliberately NOT in the _act
                # chain: the evict waits on the group's stop-matmul, and
                # chaining it would head-of-line block the next group's
                # first ACT phase behind that wait.
                for h in range(W // HALF):
                    nc.scalar.activation(res[:, h * HALF:(h + 1) * HALF],
                                         acc[h][:], AF.Identity, bias=endC)
                nc.sync.dma_start(out[:, cs], res[:])
    nc.finalize()
    return nc


def _host_params(cx, cy, w, h, Pw, A, a, B_off, lx, ly, rows):
    """Per-core [128, NPAR] parameter matrix (per-partition scalars)."""
    pr = np.zeros((P, NPAR), dtype=np.float32)
    for i in range(NCHIP):
        la = a * lx[i]
        ha = a * ly[i]
        W0 = 0.5 * w[rows, i] / la
        H0 = 0.5 * h[rows, i] / ha
        cxl = cx[rows, i] / la
        cyl = cy[rows, i] / ha
        pr[:, 4 * i + 0] = np.repeat(W0 + cxl, REP)   # b'm = -x/(a lx) + .
        pr[:, 4 * i + 1] = np.repeat(W0 - cxl, REP)   # b'p = +x/(a lx) + .
        pr[:, 4 * i + 2] = np.repeat(H0 + cyl, REP)
        pr[:, 4 * i + 3] = np.repeat(H0 - cyl, REP)
    pr[:, 4 * NCHIP] = np.repeat(A * B_off * Pw[rows].sum(axis=1), REP)
    return np.ascontiguousarray(pr, dtype=np.float32)


def _host_diag(Pw, A, a, rows):
    """[128, NCHIP*128] fp32: per chiplet a diag(-Pi*A*a*2/sqrt(pi))."""
    dg = np.zeros((P, NCHIP * P), dtype=np.float32)
    idx = np.arange(P)
    for i in range(NCHIP):
        s = np.repeat(-C1 * A * a * Pw[rows, i], REP).astype(np.float32)
        dg[idx, i * P + idx] = s
    return np.ascontiguousarray(dg)


_CACHE = {}


def _get_executor(scal):
    """Build (once) the Bass program and a cached jitted SPMD callable.

    The callable takes the device-resident inputs as [8·128, ...] arrays
    sharded over 8 cores; the output scratch buffer is created on-device
    inside the same jit call (no host transfer)."""
    if "exec" in _CACHE:
        return _CACHE["exec"]

    import jax
    import jax.numpy as jnp
    from jax.sharding import Mesh, NamedSharding, PartitionSpec
    from jax.experimental.shard_map import shard_map
    from concourse import bass2jax
    import concourse.mybir as mybir

    nc = _build_program(scal)
    _CACHE["nc"] = nc
    bass2jax.install_neuronx_cc_hook()

    partition_name = (nc.partition_id_tensor.name
                      if nc.partition_id_tensor else None)
    in_names, out_names, out_avals = [], [], []
    for alloc in nc.m.functions[0].allocations:
        if not isinstance(alloc, mybir.MemoryLocationSet):
            continue
        name = alloc.memorylocations[0].name
        if alloc.kind == "ExternalInput":
            if name != partition_name:
                in_names.append(name)
        elif alloc.kind == "ExternalOutput":
            out_names.append(name)
            out_avals.append(jax.core.ShapedArray(
                tuple(alloc.tensor_shape), mybir.dt.np(alloc.dtype)))
    n_params = len(in_names)
    all_names = in_names + out_names
    if partition_name is not None:
        all_names = all_names + [partition_name]

    def _body(*args):
        # args = real inputs + one dummy buffer per output. On this
        # (axon/PJRT) path the output operands are inert: the NEFF rename
        # binds the bass "out" tensor to the custom-call RESULT buffer, so
        # the dummy is never read -- it only satisfies the hook's
        # param-order check. One persistent buffer is reused every call.
        operands = list(args)
        if partition_name is not None:
            operands.append(bass2jax.partition_id_tensor())
        outs = bass2jax._bass_exec_p.bind(
            *operands,
            out_avals=tuple(out_avals),
            in_names=tuple(all_names),
            out_names=tuple(out_names),
            lowering_input_output_aliases=(),
            sim_require_finite=True,
            sim_require_nnan=True,
            nc=nc,
        )
        return tuple(outs)

    devices = jax.devices()[:N_CORES]
    mesh = Mesh(np.asarray(devices), ("core",))
    sharding = NamedSharding(mesh, PartitionSpec("core"))
    sharded = jax.jit(
        shard_map(_body, mesh=mesh,
                  in_specs=(PartitionSpec("core"),) * (n_params + len(out_avals)),
                  out_specs=(PartitionSpec("core"),) * len(out_avals),
                  check_rep=False),
        keep_unused=True)

    # device-side dummy output operands (content never read)
    zshapes = [(N_CORES * s.shape[0], *s.shape[1:]) for s in out_avals]
    zdtypes = [s.dtype for s in out_avals]

    def _mk(shape_dtype):
        shape, dtype = shape_dtype
        return jax.jit(lambda: jnp.zeros(shape, dtype),
                       out_shardings=sharding)

    zeros_fns = [_mk(sd) for sd in zip(zshapes, zdtypes)]
    ex = {"sharded": sharded, "in_names": in_names, "zeros_fns": zeros_fns,
          "sharding": sharding, "n_params": n_params}
    _CACHE["exec"] = ex
    return ex


def _scal_from_inputs(a, lx, ly):
    af = float(np.asarray(a).reshape(-1)[0])
    lxf = np.asarray(lx, dtype=np.float64)
    lyf = np.asarray(ly, dtype=np.float64)
    return {
        "inv_la": [float(1.0 / (af * lxf[i])) for i in range(NCHIP)],
        "inv_ha": [float(1.0 / (af * lyf[i])) for i in range(NCHIP)],
    }


def _device_inputs(x, y, chiplets_x, chiplets_y, chiplets_width,
                   chiplets_height, chiplets_power, A, a, B_off, lx, ly):
    """Full-input -> per-core-stacked device arrays keyed by tensor name."""
    x = np.asarray(x, dtype=np.float32)
    y = np.asarray(y, dtype=np.float32)
    cx = np.asarray(chiplets_x, dtype=np.float32)
    cy = np.asarray(chiplets_y, dtype=np.float32)
    w = np.asarray(chiplets_width, dtype=np.float32)
    h = np.asarray(chiplets_height, dtype=np.float32)
    Pw = np.asarray(chiplets_power, dtype=np.float32)
    Af = float(np.asarray(A).reshape(-1)[0])
    af = float(np.asarray(a).reshape(-1)[0])
    Bf = float(np.asarray(B_off).reshape(-1)[0])
    lxf = np.asarray(lx, dtype=np.float64)
    lyf = np.asarray(ly, dtype=np.float64)

    xs = np.ascontiguousarray(x.reshape(N_CORES * P, F))
    ys = np.ascontiguousarray(y.reshape(N_CORES * P, F))
    prs = np.concatenate(
        [_host_params(cx, cy, w, h, Pw, Af, af, Bf, lxf, lyf,
                      slice(c * RPC, (c + 1) * RPC)) for c in range(N_CORES)],
        axis=0)
    dgs = np.concatenate(
        [_host_diag(Pw, Af, af, slice(c * RPC, (c + 1) * RPC))
         for c in range(N_CORES)], axis=0)
    return {"xin": xs, "yin": ys, "prm": prs, "dgf": dgs,
            "dgh": dgs.astype(np.float16)}


def run(x, y, chiplets_x, chiplets_y, chiplets_width, chiplets_height,
        chiplets_power, A, a, B_off, lx, ly, grid=None):
    import jax

    ex = _get_executor(_scal_from_inputs(a, lx, ly))
    arrs = _device_inputs(x, y, chiplets_x, chiplets_y, chiplets_width,
                          chiplets_height, chiplets_power, A, a, B_off,
                          lx, ly)
    ins = [jax.device_put(arrs[nm], ex["sharding"]) for nm in ex["in_names"]]
    scratch = [zf() for zf in ex["zeros_fns"]]
    out = ex["sharded"](*ins, *scratch)
    full = np.asarray(out[0]).reshape(B, G2).astype(np.float32, copy=False)
    return full


def kernel(**inputs):
    return run(**inputs)
